# revision 33
# baseline (speedup 1.0000x reference)
"""Trainium2 Bass kernel for nn_Decoder_32074815767263 (dense_mlp).

Math (per reference):
    enc_proj = enc_state @ W1[:512]          (B,T,H)
    dec_proj = dec_state @ W1[512:]          (B,U,H)
    hidden   = tanh(enc_proj[:,:,None,:] + dec_proj[:,None,:,:] + b1)
    logits   = hidden @ W2 + b2              (B,T,U,V)

Sharding: 8 cores = (B=4) x (T halves of 150). Each core computes its
(150, 60, 1000) output slab independently; no collectives.

Per-core pipeline (SPMD-identical program, data differs per core):
  - PE p-state warm-up (dummy matmuls on zeros) while input DMAs fly.
  - enc_projT[h,t] / dec_projT[h,u] via bf16 matmuls into fp32 PSUM
    (W1 natural layout is already the lhsT the PE wants).
  - hiddenT materialized in transposed layout [H-part, row], row = u*150+t,
    into a 3072-column circular SBUF buffer (bf16):
      DVE: hid[:, span] = enc_projT + dec_projT[:,u] + b1, batched over
           several u-tiles per instruction via stride-0 broadcast APs
      ACT: tanh in progressive groups (128/256 at ramp, 768 steady-state);
           H-chunks 0-1 additionally cast to an fp8e4 copy (hid8)
  - PE per 128-row block: 1 fp8 DoubleRow matmul per vocab half covers
    H-chunks 0-1 at 2 bytes/cycle (2x fp8 rate), then 12 bf16 matmuls
    (6 chunks x 2 vocab halves) accumulate into the same 2-bank PSUM tile
    (4 tiles in flight). Both fp8 W28 and bf16 w2t are host-prescaled by
    16 (exact in bf16; dodges e4m3 denormals for fp8), so PSUM holds 16x
    logits. rel_err ~1.8e-2 vs fp32 reference, inside the 2e-2 gate.
  - Drain: ACT scales v-half 0 by 1/16, DVE v-half 1, PSUM fp32 -> bf16
    out tile; single contiguous 250KB DMA per block into a flat
    [9000, 1000] bf16 DRAM output (u-major).
  - b2 add, bf16->fp32 cast, and (u,t)->(t,u) transpose happen on host.

DMA queues: sync HWDGE carries W1enc + 3 W2 chunks then all output stores;
scalar HWDGE carries encT/decT/b1/W1dec + W28 + 3 W2 chunks.
"""

import sys

for _p in ("/opt/trn_rl_repo", "/root/.axon_site/_ro/trn_rl_repo"):
    if _p not in sys.path:
        sys.path.append(_p)

import ml_dtypes
import numpy as np

_B, _T, _U = 4, 300, 60
_D, _H, _V = 512, 1024, 1000
_TC = 150                      # T rows per core
_ROWS = _TC * _U               # 9000 hidden rows per core
_CB = 3072                     # circular hid buffer columns (multiple of 128 and 150's lcm window)
_NBLK = (_ROWS + 127) // 128   # 71 matmul row-blocks

_PROGRAM = None


def _build_program():
    from contextlib import ExitStack

    import concourse.bass as bass
    import concourse.tile as tile
    from concourse import bacc, mybir

    f32 = mybir.dt.float32
    bf16 = mybir.dt.bfloat16
    fp8 = mybir.dt.float8e4
    Tanh = mybir.ActivationFunctionType.Tanh
    DoubleRow = mybir.MatmulPerfMode.DoubleRow
    Alu = mybir.AluOpType

    nc = bacc.Bacc("TRN2", target_bir_lowering=False, debug=False)

    encT = nc.dram_tensor("encT", [_D, _TC], bf16, kind="ExternalInput")
    decT = nc.dram_tensor("decT", [_D, _U], bf16, kind="ExternalInput")
    W1 = nc.dram_tensor("W1", [2 * _D, _H], bf16, kind="ExternalInput")
    b1T = nc.dram_tensor("b1T", [128, 8], f32, kind="ExternalInput")
    W2 = nc.dram_tensor("W2", [_H, _V], bf16, kind="ExternalInput")
    W28 = nc.dram_tensor("W28", [128, 2, _V], fp8, kind="ExternalInput")
    out = nc.dram_tensor("out", [_ROWS, _V], bf16, kind="ExternalOutput")

    with ExitStack() as ctx:
        tc = ctx.enter_context(tile.TileContext(nc))
        consts = ctx.enter_context(tc.tile_pool(name="consts", bufs=1))
        outp = ctx.enter_context(tc.tile_pool(name="outp", bufs=8))
        psmain = ctx.enter_context(tc.tile_pool(name="psmain", bufs=3, space="PSUM"))
        psproj = ctx.enter_context(tc.tile_pool(name="psproj", bufs=2, space="PSUM"))

        w1t = consts.tile([128, 8, _H], bf16, tag="w1t")
        w2t = consts.tile([128, 6, _V], bf16, tag="w2t")
        w28 = consts.tile([128, 2, _V], fp8, tag="w28")
        encTs = consts.tile([128, 4, _TC], bf16, tag="encTs")
        decTs = consts.tile([128, 4, _U], bf16, tag="decTs")
        b1s = consts.tile([128, 8], f32, tag="b1s")
        # bf16 so the DVE pre-activation adds qualify for the 2x/4x perf modes
        # (all non-scalar operands 2-byte, packed; [P,1] scalars are exempt)
        encp = consts.tile([128, 8, _TC], bf16, tag="encp")
        dpb = consts.tile([128, 8, _U], f32, tag="dpb")
        hid = consts.tile([128, 8, _CB], bf16, tag="hid")
        hid8 = consts.tile([128, 2, _CB], fp8, tag="hid8")

        W1r = W1[:].rearrange("(c p) h -> p c h", p=128)
        W2r = W2[:].rearrange("(c p) v -> p c v", p=128)
        # two HW DMA queues; W1/enc/dec first (they gate the whole pipeline),
        # W2 last split across both queues (needed only once block 0's hid
        # rows are tanh'd).
        nc.scalar.dma_start(out=encTs[:], in_=encT[:].rearrange("(c p) t -> p c t", p=128))
        for d in range(4):
            nc.sync.dma_start(out=w1t[:, d, :], in_=W1r[:, d, :])
        nc.scalar.dma_start(out=decTs[:], in_=decT[:].rearrange("(c p) t -> p c t", p=128))
        nc.scalar.dma_start(out=b1s[:], in_=b1T[:])
        for d in range(4, 8):
            nc.scalar.dma_start(out=w1t[:, d, :], in_=W1r[:, d, :])

        # ---- PE p-state warm-up: harmless matmuls on zeros while the input
        # DMAs are in flight, so the first real matmuls run at full clock ----
        zt = consts.tile([128, 512], bf16, tag="zt")
        nc.vector.memset(zt[:], 0)
        pswu = psmain.tile([128, 2, 512], f32, tag="ps")
        for i in range(10):
            nc.tensor.matmul(
                zt_out := pswu[:, i % 2, :],
                zt[:, 0:128],
                zt[:],
                start=True,
                stop=True,
                skip_group_check=True,
            )

        # ---- projections, interleaved per h-chunk so the first adds/tanh/
        # matmuls (which depend per-h) can start before all h are projected ----
        for h in range(8):
            pset = psproj.tile([128, 512], f32, tag="pp")
            pse = pset[:, : _TC]
            for d in range(4):
                nc.tensor.matmul(
                    pse[:],
                    w1t[:, d, 128 * h : 128 * (h + 1)],
                    encTs[:, d, :],
                    start=(d == 0),
                    stop=(d == 3),
                )
            if h % 2 == 0:
                nc.scalar.copy(out=encp[:, h, :], in_=pse[:])
            else:
                nc.vector.tensor_copy(out=encp[:, h, :], in_=pse[:])
            psd = pset[:, 256:320]
            for d in range(4):
                nc.tensor.matmul(
                    psd[:, : _U],
                    w1t[:, 4 + d, 128 * h : 128 * (h + 1)],
                    decTs[:, d, :],
                    start=(d == 0),
                    stop=(d == 3),
                )
            if h % 2 == 0:
                nc.scalar.add(out=dpb[:, h, :], in_=psd[:, : _U], add=b1s[:, h : h + 1])
            else:
                nc.vector.tensor_scalar_add(
                    out=dpb[:, h, :], in0=psd[:, : _U], scalar1=b1s[:, h : h + 1]
                )
            # u0/u1 adds for this h right away: block 0/1's per-h pipeline
            # starts before the remaining h-chunks are even projected
            for u in range(2):
                nc.vector.tensor_scalar_add(
                    out=hid[:, h, _TC * u : _TC * (u + 1)],
                    in0=encp[:, h, :],
                    scalar1=dpb[:, h, u : u + 1],
                )

        # W2 group loads only now: W1 owned the HBM bandwidth during the
        # projection ramp; these 2.2MB land in the idle window before block 0
        # consumes them (w28 first -- the DR matmul opens every block).
        nc.sync.dma_start(out=w28[:], in_=W28[:])
        for c in range(3):
            nc.sync.dma_start(out=w2t[:, c, :], in_=W2r[:, 2 + c, :])
        for c in range(3, 6):
            nc.scalar.dma_start(out=w2t[:, c, :], in_=W2r[:, 2 + c, :])

        # ---- batched pre-activation adds: one DVE tensor_tensor per h covers
        # nu u-tiles via stride-0 broadcast APs (encp along u, dpb along t) ----
        def emit_add_batch(u0, nu):
            off = (_TC * u0) % _CB
            assert off + nu * _TC <= _CB
            for h in range(8):
                o = hid[:, h, off : off + nu * _TC].rearrange("p (u t) -> p u t", u=nu)
                a = encp[:, h, :].rearrange("p (o t) -> p o t", o=1)
                b = dpb[:, h, u0 : u0 + nu].rearrange("p (u o) -> p u o", o=1)
                ab, bb = bass.broadcast_tensor_aps(a, b)
                nc.vector.tensor_add(out=o, in0=ab, in1=bb)

        # ---- pre-activation adds (DVE, per-u tensor_scalar: 2x/4x eligible) ----
        def emit_add(u):
            off = (_TC * u) % _CB
            L = min(_TC, _CB - off)
            for h in range(8):
                nc.vector.tensor_scalar_add(
                    out=hid[:, h, off : off + L],
                    in0=encp[:, h, 0:L],
                    scalar1=dpb[:, h, u : u + 1],
                )
                if L < _TC:
                    nc.vector.tensor_scalar_add(
                        out=hid[:, h, 0 : _TC - L],
                        in0=encp[:, h, L:_TC],
                        scalar1=dpb[:, h, u : u + 1],
                    )

        # ---- tanh over row range [a, b) of the circular buffer (ACT) ----
        def emit_tanh(a, b):
            while a < b:
                c0 = a % _CB
                L = min(b - a, _CB - c0)
                for h in range(2):
                    nc.scalar.activation(
                        out=hid8[:, h, c0 : c0 + L],
                        in_=hid[:, h, c0 : c0 + L],
                        func=Tanh,
                    )
                for h in range(2, 8):
                    nc.scalar.activation(
                        out=hid[:, h, c0 : c0 + L],
                        in_=hid[:, h, c0 : c0 + L],
                        func=Tanh,
                    )
                a += L

        # ---- per-block matmul + split drain + contiguous store ----
        def emit_block(k):
            r0 = 128 * k
            M = min(128, _ROWS - r0)
            c0 = r0 % _CB
            ps = psmain.tile([128, 2, 512], f32, tag="ps")
            # fp8 DoubleRow pair for H-chunks 0-1 at 2x rate (<=64 out rows
            # per matmul), opening the accumulation (start=True resets PSUM);
            # bf16 chunks accumulate on top. Both sides are host-prescaled by
            # 16 (W28 fp8 to dodge e4m3 denormals; w2t bf16 exactly), so the
            # shared PSUM holds 16x logits and the drain scales by 1/16.
            for v in range(2):
                nc.tensor.matmul(
                    ps[:M, v, 0:500],
                    hid8[:, 0:2, c0 : c0 + M],
                    w28[:, 0:2, 500 * v : 500 * (v + 1)],
                    start=True,
                    stop=False,
                    perf_mode=DoubleRow,
                    skip_group_check=True,
                )
            # h outer: both v-halves of an h share the stationary operand and
            # depend only on that h's slice of hid (faster pipeline at ramp)
            for h in range(2, 8):
                for v in range(2):
                    nc.tensor.matmul(
                        ps[:M, v, 0:500],
                        hid[:, h, c0 : c0 + M],
                        w2t[:, h - 2, 500 * v : 500 * (v + 1)],
                        start=False,
                        stop=(h == 7),
                        skip_group_check=True,
                    )
            pending.append((k, r0, M, ps))

        def flush_drains(upto=None):
            while pending and (upto is None or pending[0][0] <= upto):
                _, r0, M, ps = pending.pop(0)
                ot = outp.tile([128, _V], bf16, tag="ot")
                nc.scalar.mul(out=ot[:M, 0:500], in_=ps[:M, 0, 0:500], mul=0.0625)
                nc.vector.tensor_scalar_mul(
                    out=ot[:M, 500:1000], in0=ps[:M, 1, 0:500], scalar1=0.0625
                )
                nc.sync.dma_start(out=out[r0 : r0 + M, :], in_=ot[:M, :])

        # ---- main loop: produce u-tiles, tanh progressive groups (small at
        # ramp for latency, 768 steady-state for low ACT overhead), consume
        # completed 128-row blocks ----
        bounds = [128, 256, 512]
        while bounds[-1] < _ROWS:
            bounds.append(min(bounds[-1] + 768, _ROWS))
        next_blk = 0
        tanh_done = 0
        bi = 0
        pending = []
        emit_tanh(0, 256)
        tanh_done = 256
        bi = 2
        emit_block(0)
        emit_block(1)
        next_blk = 2
        u = 2
        while u < _U:
            off = (_TC * u) % _CB
            nu = min(3 if u < 12 else 5, _U - u, (_CB - off) // _TC)
            if nu >= 2:
                emit_add_batch(u, nu)
            else:
                emit_add(u)
                nu = 1
            u += nu
            # drains for blocks emitted last round: their matmuls are long
            # done by now, so these never stall the ACT/DVE queues
            flush_drains()
            done = _TC * u
            while bi < len(bounds) and bounds[bi] <= done:
                emit_tanh(tanh_done, bounds[bi])
                tanh_done = bounds[bi]
                bi += 1
                while next_blk < _NBLK and min(128 * (next_blk + 1), _ROWS) <= tanh_done:
                    emit_block(next_blk)
                    next_blk += 1
        flush_drains()
        assert next_blk == _NBLK and tanh_done == _ROWS, (next_blk, tanh_done)

    nc.finalize()
    return nc


def _get_program():
    global _PROGRAM
    if _PROGRAM is None:
        _PROGRAM = _build_program()
    return _PROGRAM


def _make_in_maps(enc, dec, W1, b1, W2, b2):
    bf = ml_dtypes.bfloat16
    b1T = np.ascontiguousarray(b1.reshape(8, 128).T)
    W1b = W1.astype(bf)
    W2b = (16.0 * W2).astype(bf)
    W28 = np.ascontiguousarray(
        (16.0 * W2[0:256]).astype(ml_dtypes.float8_e4m3fn).reshape(2, 128, _V).transpose(1, 0, 2)
    )
    in_maps = []
    for c in range(8):
        b, half = divmod(c, 2)
        in_maps.append(
            {
                "encT": np.ascontiguousarray(enc[b, half * _TC : (half + 1) * _TC, :].T.astype(bf)),
                "decT": np.ascontiguousarray(dec[b].T.astype(bf)),
                "W1": W1b,
                "b1T": b1T,
                "W2": W2b,
                "W28": W28,
            }
        )
    return in_maps


def kernel(enc_state, dec_state, W1, b1, W2, b2):
    from concourse.bass_utils import run_bass_kernel_spmd

    enc = np.ascontiguousarray(np.asarray(enc_state, dtype=np.float32))
    dec = np.ascontiguousarray(np.asarray(dec_state, dtype=np.float32))
    W1 = np.ascontiguousarray(np.asarray(W1, dtype=np.float32))
    b1 = np.ascontiguousarray(np.asarray(b1, dtype=np.float32))
    W2 = np.ascontiguousarray(np.asarray(W2, dtype=np.float32))
    b2 = np.ascontiguousarray(np.asarray(b2, dtype=np.float32))

    nc = _get_program()
    in_maps = _make_in_maps(enc, dec, W1, b1, W2, b2)
    res = run_bass_kernel_spmd(nc, in_maps, list(range(8)))

    full = np.empty((_B, _T, _U, _V), np.float32)
    for c in range(8):
        b, half = divmod(c, 2)
        # device output is flat [rows=9000, V] bf16 with row = u*150 + t
        o = np.asarray(res.results[c]["out"]).reshape(_U, _TC, _V)
        full[b, half * _TC : (half + 1) * _TC] = o.transpose(1, 0, 2)
    full += b2
    return full


# revision 35
# speedup vs baseline: 1.0058x; 1.0058x over previous
"""Trainium2 Bass kernel for nn_Decoder_32074815767263 (dense_mlp).

Math (per reference):
    enc_proj = enc_state @ W1[:512]          (B,T,H)
    dec_proj = dec_state @ W1[512:]          (B,U,H)
    hidden   = tanh(enc_proj[:,:,None,:] + dec_proj[:,None,:,:] + b1)
    logits   = hidden @ W2 + b2              (B,T,U,V)

Sharding: 8 cores = (B=4) x (T halves of 150). Each core computes its
(150, 60, 1000) output slab independently; no collectives.

Per-core pipeline (SPMD-identical program, data differs per core):
  - PE p-state warm-up (dummy matmuls on zeros) while input DMAs fly.
  - enc_projT[h,t] / dec_projT[h,u] via bf16 matmuls into fp32 PSUM
    (W1 natural layout is already the lhsT the PE wants).
  - hiddenT materialized in transposed layout [H-part, row], row = u*150+t,
    into a 3072-column circular SBUF buffer (bf16):
      DVE: hid[:, span] = enc_projT + dec_projT[:,u] + b1, batched over
           several u-tiles per instruction via stride-0 broadcast APs
      ACT: tanh in progressive groups (128/256 at ramp, 768 steady-state);
           H-chunks 0-1 additionally cast to an fp8e4 copy (hid8)
  - PE per 128-row block: 1 fp8 DoubleRow matmul per vocab half covers
    H-chunks 0-1 at 2 bytes/cycle (2x fp8 rate), then 12 bf16 matmuls
    (6 chunks x 2 vocab halves) accumulate into the same 2-bank PSUM tile
    (4 tiles in flight). Both fp8 W28 and bf16 w2t are host-prescaled by
    16 (exact in bf16; dodges e4m3 denormals for fp8), so PSUM holds 16x
    logits. rel_err ~1.8e-2 vs fp32 reference, inside the 2e-2 gate.
  - Drain: ACT scales v-half 0 by 1/16, DVE v-half 1, PSUM fp32 -> bf16
    out tile; single contiguous 250KB DMA per block into a flat
    [9000, 1000] bf16 DRAM output (u-major).
  - b2 add, bf16->fp32 cast, and (u,t)->(t,u) transpose happen on host.

DMA queues: sync HWDGE carries W1enc + 3 W2 chunks then all output stores;
scalar HWDGE carries encT/decT/b1/W1dec + W28 + 3 W2 chunks.
"""

import sys

for _p in ("/opt/trn_rl_repo", "/root/.axon_site/_ro/trn_rl_repo"):
    if _p not in sys.path:
        sys.path.append(_p)

import ml_dtypes
import numpy as np

_B, _T, _U = 4, 300, 60
_D, _H, _V = 512, 1024, 1000
_TC = 150                      # T rows per core
_ROWS = _TC * _U               # 9000 hidden rows per core
_CB = 3072                     # circular hid buffer columns (multiple of 128 and 150's lcm window)
_NBLK = (_ROWS + 127) // 128   # 71 matmul row-blocks

_PROGRAM = None


def _build_program():
    from contextlib import ExitStack

    import concourse.bass as bass
    import concourse.tile as tile
    from concourse import bacc, mybir

    f32 = mybir.dt.float32
    bf16 = mybir.dt.bfloat16
    fp8 = mybir.dt.float8e4
    Tanh = mybir.ActivationFunctionType.Tanh
    DoubleRow = mybir.MatmulPerfMode.DoubleRow
    Alu = mybir.AluOpType

    nc = bacc.Bacc("TRN2", target_bir_lowering=False, debug=False)

    encT = nc.dram_tensor("encT", [_D, _TC], bf16, kind="ExternalInput")
    decT = nc.dram_tensor("decT", [_D, _U], bf16, kind="ExternalInput")
    W1 = nc.dram_tensor("W1", [2 * _D, _H], bf16, kind="ExternalInput")
    b1T = nc.dram_tensor("b1T", [128, 8], f32, kind="ExternalInput")
    W2 = nc.dram_tensor("W2", [_H, _V], bf16, kind="ExternalInput")
    W28 = nc.dram_tensor("W28", [128, 2, _V], fp8, kind="ExternalInput")
    out = nc.dram_tensor("out", [_ROWS, _V], bf16, kind="ExternalOutput")

    with ExitStack() as ctx:
        tc = ctx.enter_context(tile.TileContext(nc))
        consts = ctx.enter_context(tc.tile_pool(name="consts", bufs=1))
        outp = ctx.enter_context(tc.tile_pool(name="outp", bufs=8))
        psmain = ctx.enter_context(tc.tile_pool(name="psmain", bufs=3, space="PSUM"))
        psproj = ctx.enter_context(tc.tile_pool(name="psproj", bufs=2, space="PSUM"))

        w1t = consts.tile([128, 8, _H], bf16, tag="w1t")
        w2t = consts.tile([128, 6, _V], bf16, tag="w2t")
        w28 = consts.tile([128, 2, _V], fp8, tag="w28")
        encTs = consts.tile([128, 4, _TC], bf16, tag="encTs")
        decTs = consts.tile([128, 4, _U], bf16, tag="decTs")
        b1s = consts.tile([128, 8], f32, tag="b1s")
        # bf16 so the DVE pre-activation adds qualify for the 2x/4x perf modes
        # (all non-scalar operands 2-byte, packed; [P,1] scalars are exempt)
        encp = consts.tile([128, 8, _TC], bf16, tag="encp")
        dpb = consts.tile([128, 8, _U], f32, tag="dpb")
        hid = consts.tile([128, 8, _CB], bf16, tag="hid")
        hid8 = consts.tile([128, 2, _CB], fp8, tag="hid8")

        W1r = W1[:].rearrange("(c p) h -> p c h", p=128)
        W2r = W2[:].rearrange("(c p) v -> p c v", p=128)
        # two HW DMA queues; W1/enc/dec first (they gate the whole pipeline),
        # W2 last split across both queues (needed only once block 0's hid
        # rows are tanh'd).
        nc.scalar.dma_start(out=encTs[:], in_=encT[:].rearrange("(c p) t -> p c t", p=128))
        for d in range(4):
            nc.sync.dma_start(out=w1t[:, d, :], in_=W1r[:, d, :])
        nc.scalar.dma_start(out=decTs[:], in_=decT[:].rearrange("(c p) t -> p c t", p=128))
        nc.scalar.dma_start(out=b1s[:], in_=b1T[:])
        for d in range(4, 8):
            nc.scalar.dma_start(out=w1t[:, d, :], in_=W1r[:, d, :])

        # ---- PE p-state warm-up: harmless matmuls on zeros while the input
        # DMAs are in flight, so the first real matmuls run at full clock ----
        zt = consts.tile([128, 512], bf16, tag="zt")
        nc.vector.memset(zt[:], 0)
        pswu = psmain.tile([128, 2, 512], f32, tag="ps")
        for i in range(10):
            nc.tensor.matmul(
                zt_out := pswu[:, i % 2, :],
                zt[:, 0:128],
                zt[:],
                start=True,
                stop=True,
                skip_group_check=True,
            )

        # ---- projections, interleaved per h-chunk so the first adds/tanh/
        # matmuls (which depend per-h) can start before all h are projected ----
        for h in range(8):
            pset = psproj.tile([128, 512], f32, tag="pp")
            pse = pset[:, : _TC]
            for d in range(4):
                nc.tensor.matmul(
                    pse[:],
                    w1t[:, d, 128 * h : 128 * (h + 1)],
                    encTs[:, d, :],
                    start=(d == 0),
                    stop=(d == 3),
                )
            if h % 2 == 0:
                nc.scalar.copy(out=encp[:, h, :], in_=pse[:])
            else:
                nc.vector.tensor_copy(out=encp[:, h, :], in_=pse[:])
            psd = pset[:, 256:320]
            for d in range(4):
                nc.tensor.matmul(
                    psd[:, : _U],
                    w1t[:, 4 + d, 128 * h : 128 * (h + 1)],
                    decTs[:, d, :],
                    start=(d == 0),
                    stop=(d == 3),
                )
            if h % 2 == 0:
                nc.scalar.add(out=dpb[:, h, :], in_=psd[:, : _U], add=b1s[:, h : h + 1])
            else:
                nc.vector.tensor_scalar_add(
                    out=dpb[:, h, :], in0=psd[:, : _U], scalar1=b1s[:, h : h + 1]
                )
            # u0/u1 adds for this h right away: block 0/1's per-h pipeline
            # starts before the remaining h-chunks are even projected
            for u in range(2):
                nc.vector.tensor_scalar_add(
                    out=hid[:, h, _TC * u : _TC * (u + 1)],
                    in0=encp[:, h, :],
                    scalar1=dpb[:, h, u : u + 1],
                )

        # W2 group loads only now: W1 owned the HBM bandwidth during the
        # projection ramp; these 2.2MB land in the idle window before block 0
        # consumes them (w28 first -- the DR matmul opens every block).
        nc.sync.dma_start(out=w28[:], in_=W28[:])
        for c in range(3):
            nc.sync.dma_start(out=w2t[:, c, :], in_=W2r[:, 2 + c, :])
        for c in range(3, 6):
            nc.scalar.dma_start(out=w2t[:, c, :], in_=W2r[:, 2 + c, :])

        # ---- batched pre-activation adds: one DVE tensor_tensor per h covers
        # nu u-tiles via stride-0 broadcast APs (encp along u, dpb along t) ----
        def emit_add_batch(u0, nu):
            off = (_TC * u0) % _CB
            assert off + nu * _TC <= _CB
            for h in range(8):
                o = hid[:, h, off : off + nu * _TC].rearrange("p (u t) -> p u t", u=nu)
                a = encp[:, h, :].rearrange("p (o t) -> p o t", o=1)
                b = dpb[:, h, u0 : u0 + nu].rearrange("p (u o) -> p u o", o=1)
                ab, bb = bass.broadcast_tensor_aps(a, b)
                nc.vector.tensor_add(out=o, in0=ab, in1=bb)

        # ---- pre-activation adds (DVE, per-u tensor_scalar: 2x/4x eligible) ----
        def emit_add(u):
            off = (_TC * u) % _CB
            L = min(_TC, _CB - off)
            for h in range(8):
                nc.vector.tensor_scalar_add(
                    out=hid[:, h, off : off + L],
                    in0=encp[:, h, 0:L],
                    scalar1=dpb[:, h, u : u + 1],
                )
                if L < _TC:
                    nc.vector.tensor_scalar_add(
                        out=hid[:, h, 0 : _TC - L],
                        in0=encp[:, h, L:_TC],
                        scalar1=dpb[:, h, u : u + 1],
                    )

        # ---- tanh over row range [a, b) of the circular buffer (ACT) ----
        def emit_tanh(a, b):
            while a < b:
                c0 = a % _CB
                L = min(b - a, _CB - c0)
                for h in range(2):
                    nc.scalar.activation(
                        out=hid8[:, h, c0 : c0 + L],
                        in_=hid[:, h, c0 : c0 + L],
                        func=Tanh,
                    )
                for h in range(2, 8):
                    nc.scalar.activation(
                        out=hid[:, h, c0 : c0 + L],
                        in_=hid[:, h, c0 : c0 + L],
                        func=Tanh,
                    )
                a += L

        # ---- per-block matmul + split drain + contiguous store ----
        def emit_block(k):
            r0 = 128 * k
            M = min(128, _ROWS - r0)
            c0 = r0 % _CB
            ps = psmain.tile([128, 2, 512], f32, tag="ps")
            # fp8 DoubleRow pair for H-chunks 0-1 at 2x rate (<=64 out rows
            # per matmul), opening the accumulation (start=True resets PSUM);
            # bf16 chunks accumulate on top. Both sides are host-prescaled by
            # 16 (W28 fp8 to dodge e4m3 denormals; w2t bf16 exactly), so the
            # shared PSUM holds 16x logits and the drain scales by 1/16.
            for v in range(2):
                nc.tensor.matmul(
                    ps[:M, v, 0:500],
                    hid8[:, 0:2, c0 : c0 + M],
                    w28[:, 0:2, 500 * v : 500 * (v + 1)],
                    start=True,
                    stop=False,
                    perf_mode=DoubleRow,
                    skip_group_check=True,
                )
            # h outer: both v-halves of an h share the stationary operand and
            # depend only on that h's slice of hid (faster pipeline at ramp)
            for h in range(2, 8):
                for v in range(2):
                    nc.tensor.matmul(
                        ps[:M, v, 0:500],
                        hid[:, h, c0 : c0 + M],
                        w2t[:, h - 2, 500 * v : 500 * (v + 1)],
                        start=False,
                        stop=(h == 7),
                        skip_group_check=True,
                    )
            pending.append((k, r0, M, ps))

        def flush_drains(upto=None):
            while pending and (upto is None or pending[0][0] <= upto):
                _, r0, M, ps = pending.pop(0)
                ot = outp.tile([128, _V], bf16, tag="ot")
                nc.scalar.mul(out=ot[:M, 0:500], in_=ps[:M, 0, 0:500], mul=0.0625)
                nc.vector.tensor_scalar_mul(
                    out=ot[:M, 500:1000], in0=ps[:M, 1, 0:500], scalar1=0.0625
                )
                nc.sync.dma_start(out=out[r0 : r0 + M, :], in_=ot[:M, :])

        # ---- main loop: produce u-tiles, tanh progressive groups (small at
        # ramp for latency, 768 steady-state for low ACT overhead), consume
        # completed 128-row blocks ----
        bounds = [128, 256, 512]
        while bounds[-1] < _ROWS:
            bounds.append(min(bounds[-1] + 768, _ROWS))
        next_blk = 0
        tanh_done = 0
        bi = 0
        pending = []
        emit_tanh(0, 256)
        tanh_done = 256
        bi = 2
        emit_block(0)
        emit_block(1)
        next_blk = 2
        u = 2
        while u < _U:
            off = (_TC * u) % _CB
            nu = min(3 if u < 12 else 5, _U - u, (_CB - off) // _TC)
            if nu >= 2:
                emit_add_batch(u, nu)
            else:
                emit_add(u)
                nu = 1
            u += nu
            # drains for blocks emitted last round: their matmuls are long
            # done by now, so these never stall the ACT/DVE queues
            flush_drains()
            done = _TC * u
            while bi < len(bounds) and bounds[bi] <= done:
                emit_tanh(tanh_done, bounds[bi])
                tanh_done = bounds[bi]
                bi += 1
                while next_blk < _NBLK and min(128 * (next_blk + 1), _ROWS) <= tanh_done:
                    emit_block(next_blk)
                    next_blk += 1
        flush_drains()
        assert next_blk == _NBLK and tanh_done == _ROWS, (next_blk, tanh_done)

    nc.finalize()
    return nc


def _get_program():
    global _PROGRAM
    if _PROGRAM is None:
        _PROGRAM = _build_program()
    return _PROGRAM


def _make_in_maps(enc, dec, W1, b1, W2, b2):
    bf = ml_dtypes.bfloat16
    b1T = np.ascontiguousarray(b1.reshape(8, 128).T)
    W1b = W1.astype(bf)
    W2b = (16.0 * W2).astype(bf)
    W28 = np.ascontiguousarray(
        (16.0 * W2[0:256]).astype(ml_dtypes.float8_e4m3fn).reshape(2, 128, _V).transpose(1, 0, 2)
    )
    in_maps = []
    for c in range(8):
        b, half = divmod(c, 2)
        in_maps.append(
            {
                "encT": np.ascontiguousarray(enc[b, half * _TC : (half + 1) * _TC, :].T.astype(bf)),
                "decT": np.ascontiguousarray(dec[b].T.astype(bf)),
                "W1": W1b,
                "b1T": b1T,
                "W2": W2b,
                "W28": W28,
            }
        )
    return in_maps


def kernel(enc_state, dec_state, W1, b1, W2, b2):
    from concourse.bass_utils import run_bass_kernel_spmd

    enc = np.ascontiguousarray(np.asarray(enc_state, dtype=np.float32))
    dec = np.ascontiguousarray(np.asarray(dec_state, dtype=np.float32))
    W1 = np.ascontiguousarray(np.asarray(W1, dtype=np.float32))
    b1 = np.ascontiguousarray(np.asarray(b1, dtype=np.float32))
    W2 = np.ascontiguousarray(np.asarray(W2, dtype=np.float32))
    b2 = np.ascontiguousarray(np.asarray(b2, dtype=np.float32))

    nc = _get_program()
    in_maps = _make_in_maps(enc, dec, W1, b1, W2, b2)
    res = run_bass_kernel_spmd(nc, in_maps, list(range(8)))

    full = np.empty((_B, _T, _U, _V), np.float32)
    for c in range(8):
        b, half = divmod(c, 2)
        # device output is flat [rows=9000, V] bf16 with row = u*150 + t
        o = np.asarray(res.results[c]["out"]).reshape(_U, _TC, _V)
        full[b, half * _TC : (half + 1) * _TC] = o.transpose(1, 0, 2)
    full += b2
    return full
